# revision 4
# baseline (speedup 1.0000x reference)
"""DifferentiableHungarianLoss kernel for 8 TRN2 NeuronCores.

reference semantics:
    A = latent[0], B = latent[1]                       # [512, 512] each
    cost[i, j] = ||A_i - B_j||_2                       # [512, 512] cdist
    P = Hungarian(cost)  (exact LAP, via host callback in the reference too)
    loss = |sum(P * cost) - trace(cost)| / 512
    returns (loss, arange(512), argmax(P, axis=1))

Device: the cost matrix (all the tensor math) is computed on the 8 cores with
a 4x2 2D block sharding -- core k owns cost block [128 A-rows x 256 B-rows],
so each core only DMAs 768KB (A quarter 256KB + B half 512KB) instead of a
replicated 1.125MB.  The L2 norms are folded into the PSUM accumulation as two
extra contraction rows so d2 = |A_i|^2 + |B_j|^2 - 2 A.B^T comes out of the
TensorEngine directly, then ACT does sqrt on eviction.

Host: the Jonker-Volgenant solve (inherently sequential; the reference runs it
through jax.pure_callback on host as well -- it cannot lower to neuron) plus
the final scalar arithmetic.
"""

import numpy as np

N = 512
D = 512
M_BLK = 128  # A rows per core  (4-way split)
N_BLK = 256  # B rows per core  (2-way split)
N_CORES = 8

_compiled = None


def _build():
    import concourse.bass as bass
    import concourse.tile as tile
    from concourse import bacc, mybir
    from concourse import masks
    from contextlib import ExitStack

    f32 = mybir.dt.float32
    P = 128

    nc = bacc.Bacc("TRN2", target_bir_lowering=False, debug=False,
                   num_devices=N_CORES)

    a_dram = nc.dram_tensor("a", [M_BLK, D], f32, kind="ExternalInput").ap()
    b_dram = nc.dram_tensor("b", [N_BLK, D], f32, kind="ExternalInput").ap()
    cost_dram = nc.dram_tensor("cost", [M_BLK, N_BLK], f32,
                               kind="ExternalOutput").ap()

    with tile.TileContext(nc) as tc, ExitStack() as ctx:
        const_pool = ctx.enter_context(tc.tile_pool(name="const", bufs=1))
        in_pool = ctx.enter_context(tc.tile_pool(name="inp", bufs=1))
        tp_pool = ctx.enter_context(tc.tile_pool(name="tp", bufs=1))
        sq_pool = ctx.enter_context(tc.tile_pool(name="sq", bufs=2))
        out_pool = ctx.enter_context(tc.tile_pool(name="out", bufs=1))
        ps_pool = ctx.enter_context(tc.tile_pool(name="ps", bufs=4, space="PSUM"))
        ps_small = ctx.enter_context(tc.tile_pool(name="pss", bufs=1, space="PSUM"))
        ps_acc = ctx.enter_context(tc.tile_pool(name="psacc", bufs=1, space="PSUM"))

        identity = const_pool.tile([P, P], f32)
        masks.make_identity(nc, identity[:])

        # ---- load inputs (split DMAs to spread across HW queues) ----
        a_t = in_pool.tile([P, D], f32)                 # A block, row-major
        b_t = in_pool.tile([P, 2, D], f32)              # B row r*128+p -> b_t[p, r]
        for h in range(2):
            nc.sync.dma_start(a_t[:, h * 256:(h + 1) * 256],
                              a_dram[:, h * 256:(h + 1) * 256])
        for r in range(2):
            for h in range(2):
                nc.sync.dma_start(b_t[:, r, h * 256:(h + 1) * 256],
                                  b_dram[r * 128:(r + 1) * 128,
                                         h * 256:(h + 1) * 256])

        # ---- row norms: an2[m] = sum_d A[m,d]^2, bn2[n] = sum_d B[n,d]^2 ----
        an2 = tp_pool.tile([P, 1], f32)
        sq = sq_pool.tile([P, D], f32, tag="sq")
        nc.scalar.activation(sq[:], a_t[:], mybir.ActivationFunctionType.Square,
                             accum_out=an2[:])
        bn2 = []
        for r in range(2):
            bn2_r = tp_pool.tile([P, 1], f32, tag=f"bn2_{r}")
            sq = sq_pool.tile([P, D], f32, tag="sq")
            nc.scalar.activation(sq[:], b_t[:, r, :],
                                 mybir.ActivationFunctionType.Square,
                                 accum_out=bn2_r[:])
            bn2.append(bn2_r)

        # rank-1 terms as two K=1 matmuls: an2^T x ones  and  ones x bn2^T
        # (all operands at partition 0 -- partition-offset writes are illegal)
        an2T = tp_pool.tile([1, M_BLK], f32)
        bn2T = tp_pool.tile([1, N_BLK], f32)
        ones_m = tp_pool.tile([1, M_BLK], f32)
        ones_n = tp_pool.tile([1, N_BLK], f32)
        nc.gpsimd.memset(ones_m[:], 1.0)
        nc.gpsimd.memset(ones_n[:], 1.0)
        ps = ps_small.tile([1, P], f32, tag="pnorm")
        nc.tensor.transpose(ps[:], an2[:], identity[:])
        nc.vector.tensor_copy(an2T[:], ps[:])
        for r in range(2):
            ps = ps_small.tile([1, P], f32, tag="pnorm")
            nc.tensor.transpose(ps[:], bn2[r][:], identity[:])
            nc.vector.tensor_copy(bn2T[:, r * 128:(r + 1) * 128], ps[:])

        # ---- transposes for the GEMM operands ----
        # at_sb chunk c = -2 * A^T[128c:128c+128, :]   (K on partitions)
        at_sb = tp_pool.tile([P, 4, P], f32)
        for c in range(4):
            ps = ps_pool.tile([P, P], f32, tag="ptp")
            nc.tensor.transpose(ps[:], a_t[:, c * 128:(c + 1) * 128], identity[:])
            nc.scalar.activation(at_sb[:, c], ps[:],
                                 mybir.ActivationFunctionType.Copy, scale=-2.0)
        # bt_sb chunk c = B^T[128c:128c+128, :]  ([K, 256] with K on partitions)
        bt_sb = tp_pool.tile([P, 4, N_BLK], f32)
        for c in range(4):
            for r in range(2):
                ps = ps_pool.tile([P, P], f32, tag="ptp")
                nc.tensor.transpose(ps[:], b_t[:, r, c * 128:(c + 1) * 128],
                                    identity[:])
                nc.vector.tensor_copy(bt_sb[:, c, r * 128:(r + 1) * 128], ps[:])

        # ---- d2 = an2 + bn2 - 2 A B^T accumulated in PSUM ----
        d2_ps = ps_acc.tile([P, N_BLK], f32)
        for c in range(4):
            nc.tensor.matmul(d2_ps[:], at_sb[:, c], bt_sb[:, c],
                             start=(c == 0), stop=False)
        nc.tensor.matmul(d2_ps[:], an2T[:], ones_n[:], start=False, stop=False)
        nc.tensor.matmul(d2_ps[:], ones_m[:], bn2T[:], start=False, stop=True)

        # ---- cost = sqrt(d2)  (d2 >= min-dist^2 ~ 780 for this input family) ----
        out_t = out_pool.tile([P, N_BLK], f32)
        nc.scalar.sqrt(out_t[:], d2_ps[:])
        for h in range(2):
            nc.sync.dma_start(cost_dram[:, h * 128:(h + 1) * 128],
                              out_t[:, h * 128:(h + 1) * 128])

    nc.compile()
    return nc


def _get_compiled():
    global _compiled
    if _compiled is None:
        _compiled = _build()
    return _compiled


def _run_device_cost(lat, trace=False):
    """Run the 8-core cost-matrix kernel; returns (cost [512,512] f32, results)."""
    from concourse.bass_utils import run_bass_kernel_spmd

    nc = _get_compiled()
    A = np.ascontiguousarray(lat[0], dtype=np.float32)
    B = np.ascontiguousarray(lat[1], dtype=np.float32)
    in_maps = []
    for k in range(N_CORES):
        mi, nj = k // 2, k % 2
        in_maps.append({
            "a": A[mi * M_BLK:(mi + 1) * M_BLK],
            "b": B[nj * N_BLK:(nj + 1) * N_BLK],
        })
    res = run_bass_kernel_spmd(nc, in_maps, list(range(N_CORES)), trace=trace)
    cost = np.empty((N, N), dtype=np.float32)
    for k in range(N_CORES):
        mi, nj = k // 2, k % 2
        cost[mi * M_BLK:(mi + 1) * M_BLK,
             nj * N_BLK:(nj + 1) * N_BLK] = res.results[k]["cost"]
    return cost, res


def _lap_jv(cost):
    """Exact Jonker-Volgenant LAP (dual potentials + shortest augmenting
    path); identical algorithm to the reference / scipy."""
    cost = np.asarray(cost, dtype=np.float64)
    n = cost.shape[0]
    INF = np.inf
    u = np.zeros(n + 1)
    v = np.zeros(n + 1)
    p = np.zeros(n + 1, dtype=np.int64)
    way = np.zeros(n + 1, dtype=np.int64)
    for i in range(1, n + 1):
        p[0] = i
        j0 = 0
        minv = np.full(n + 1, INF)
        used = np.zeros(n + 1, dtype=bool)
        while True:
            used[j0] = True
            i0 = p[j0]
            cur = cost[i0 - 1, :] - u[i0] - v[1:]
            free = ~used[1:]
            upd = free & (cur < minv[1:])
            minv[1:][upd] = cur[upd]
            way[1:][upd] = j0
            m = np.where(free, minv[1:], INF)
            j1 = int(np.argmin(m)) + 1
            delta = m[j1 - 1]
            iu = np.nonzero(used)[0]
            u[p[iu]] += delta
            v[iu] -= delta
            minv[1:][free] -= delta
            j0 = j1
            if p[j0] == 0:
                break
        while j0 != 0:
            j1 = way[j0]
            p[j0] = p[j1]
            j0 = j1
    col_of_row = np.empty(n, dtype=np.int64)
    col_of_row[p[1:] - 1] = np.arange(n)
    return col_of_row


def _solve_lap(cost):
    try:
        from scipy.optimize import linear_sum_assignment
        _, col = linear_sum_assignment(np.asarray(cost, dtype=np.float64))
        return col
    except Exception:
        return _lap_jv(cost)


def kernel(latent):
    lat = np.asarray(latent)
    cost, _ = _run_device_cost(lat)
    c64 = cost.astype(np.float64)
    col_ind = _solve_lap(c64)
    predicted = c64[np.arange(N), col_ind].sum()
    ideal = np.trace(c64)
    loss = np.float32(abs(predicted - ideal) / N)
    row_ind = np.arange(N, dtype=np.int32)
    return loss, row_ind, col_ind.astype(np.int32)


# revision 5
# speedup vs baseline: 1.0479x; 1.0479x over previous
"""DifferentiableHungarianLoss kernel for 8 TRN2 NeuronCores.

reference semantics:
    A = latent[0], B = latent[1]                       # [512, 512] each
    cost[i, j] = ||A_i - B_j||_2                       # [512, 512] cdist
    P = Hungarian(cost)  (exact LAP, via host callback in the reference too)
    loss = |sum(P * cost) - trace(cost)| / 512
    returns (loss, arange(512), argmax(P, axis=1))

Device: the cost matrix (all the tensor math) is computed on the 8 cores with
a 4x2 2D block sharding -- core k owns cost block [128 A-rows x 256 B-rows],
so each core only DMAs 768KB (A quarter 256KB + B half 512KB) instead of a
replicated 1.125MB.  The L2 norms are folded into the PSUM accumulation as two
extra contraction rows so d2 = |A_i|^2 + |B_j|^2 - 2 A.B^T comes out of the
TensorEngine directly, then ACT does sqrt on eviction.

Host: the Jonker-Volgenant solve (inherently sequential; the reference runs it
through jax.pure_callback on host as well -- it cannot lower to neuron) plus
the final scalar arithmetic.
"""

import numpy as np

N = 512
D = 512
M_BLK = 128  # A rows per core  (4-way split)
N_BLK = 256  # B rows per core  (2-way split)
N_CORES = 8

_compiled = None


def _build():
    import concourse.bass as bass
    import concourse.tile as tile
    from concourse import bacc, mybir
    from concourse import masks
    from contextlib import ExitStack

    f32 = mybir.dt.float32
    P = 128

    nc = bacc.Bacc("TRN2", target_bir_lowering=False, debug=False,
                   num_devices=N_CORES)

    a_dram = nc.dram_tensor("a", [M_BLK, D], f32, kind="ExternalInput").ap()
    b_dram = nc.dram_tensor("b", [N_BLK, D], f32, kind="ExternalInput").ap()
    cost_dram = nc.dram_tensor("cost", [M_BLK, N_BLK], f32,
                               kind="ExternalOutput").ap()

    with tile.TileContext(nc) as tc, ExitStack() as ctx:
        const_pool = ctx.enter_context(tc.tile_pool(name="const", bufs=1))
        in_pool = ctx.enter_context(tc.tile_pool(name="inp", bufs=1))
        tp_pool = ctx.enter_context(tc.tile_pool(name="tp", bufs=1))
        sq_pool = ctx.enter_context(tc.tile_pool(name="sq", bufs=2))
        out_pool = ctx.enter_context(tc.tile_pool(name="out", bufs=1))
        ps_pool = ctx.enter_context(tc.tile_pool(name="ps", bufs=4, space="PSUM"))
        ps_small = ctx.enter_context(tc.tile_pool(name="pss", bufs=2, space="PSUM"))
        ps_acc = ctx.enter_context(tc.tile_pool(name="psacc", bufs=1, space="PSUM"))

        identity = const_pool.tile([P, P], f32)
        masks.make_identity(nc, identity[:])

        # ---- load inputs ----
        # one dma per 128-row slab; B chunk-split by column halves so the
        # d-chunk transposes can start before the whole slab lands
        a_t = in_pool.tile([P, D], f32)                 # A block, row-major
        b_t = in_pool.tile([P, 2, D], f32)              # B row r*128+p -> b_t[p, r]
        nc.sync.dma_start(a_t[:], a_dram[:])
        for r in range(2):
            for h in range(2):
                nc.sync.dma_start(b_t[:, r, h * 256:(h + 1) * 256],
                                  b_dram[r * 128:(r + 1) * 128,
                                         h * 256:(h + 1) * 256])

        # ---- row norms ----
        # an2[m] = sum_d A[m,d]^2 stays per-partition: folded in as the bias
        # of the final sqrt activation.  bn2[n] needs to live on the free dim:
        # transpose then one rank-1 matmul into the accumulator.
        an2 = tp_pool.tile([P, 1], f32)
        sq = sq_pool.tile([P, D], f32, tag="sq")
        nc.scalar.activation(sq[:], a_t[:], mybir.ActivationFunctionType.Square,
                             accum_out=an2[:])
        bn2 = []
        for r in range(2):
            bn2_r = tp_pool.tile([P, 1], f32, tag=f"bn2_{r}")
            sq = sq_pool.tile([P, D], f32, tag="sq")
            nc.scalar.activation(sq[:], b_t[:, r, :],
                                 mybir.ActivationFunctionType.Square,
                                 accum_out=bn2_r[:])
            bn2.append(bn2_r)
        bn2T = tp_pool.tile([1, N_BLK], f32)
        ones_m = tp_pool.tile([1, M_BLK], f32)
        nc.gpsimd.memset(ones_m[:], 1.0)
        for r in range(2):
            ps = ps_small.tile([1, P], f32, tag="pnorm")
            nc.tensor.transpose(ps[:], bn2[r][:], identity[:])
            nc.vector.tensor_copy(bn2T[:, r * 128:(r + 1) * 128], ps[:])

        # ---- transposes for the GEMM operands ----
        # at_sb chunk c = -2 * A^T[128c:128c+128, :]   (K on partitions)
        at_sb = tp_pool.tile([P, 4, P], f32)
        for c in range(4):
            ps = ps_pool.tile([P, P], f32, tag="ptp")
            nc.tensor.transpose(ps[:], a_t[:, c * 128:(c + 1) * 128], identity[:])
            nc.scalar.activation(at_sb[:, c], ps[:],
                                 mybir.ActivationFunctionType.Copy, scale=-2.0)
        # bt_sb chunk c = B^T[128c:128c+128, :]  ([K, 256] with K on partitions)
        bt_sb = tp_pool.tile([P, 4, N_BLK], f32)
        for c in range(4):
            for r in range(2):
                ps = ps_pool.tile([P, P], f32, tag="ptp")
                nc.tensor.transpose(ps[:], b_t[:, r, c * 128:(c + 1) * 128],
                                    identity[:])
                nc.vector.tensor_copy(bt_sb[:, c, r * 128:(r + 1) * 128], ps[:])

        # ---- d2 - an2 = bn2 - 2 A B^T accumulated in PSUM ----
        d2_ps = ps_acc.tile([P, N_BLK], f32)
        nc.tensor.matmul(d2_ps[:], ones_m[:], bn2T[:], start=True, stop=False)
        for c in range(4):
            nc.tensor.matmul(d2_ps[:], at_sb[:, c], bt_sb[:, c],
                             start=False, stop=(c == 3))

        # ---- cost = sqrt(d2 + an2)  (an2 enters as per-partition bias) ----
        out_t = out_pool.tile([P, N_BLK], f32)
        nc.scalar.activation(out_t[:], d2_ps[:],
                             mybir.ActivationFunctionType.Sqrt, bias=an2[:])
        nc.sync.dma_start(cost_dram[:], out_t[:])

    nc.compile()
    return nc


def _get_compiled():
    global _compiled
    if _compiled is None:
        _compiled = _build()
    return _compiled


def _run_device_cost(lat, trace=False):
    """Run the 8-core cost-matrix kernel; returns (cost [512,512] f32, results)."""
    from concourse.bass_utils import run_bass_kernel_spmd

    nc = _get_compiled()
    A = np.ascontiguousarray(lat[0], dtype=np.float32)
    B = np.ascontiguousarray(lat[1], dtype=np.float32)
    in_maps = []
    for k in range(N_CORES):
        mi, nj = k // 2, k % 2
        in_maps.append({
            "a": A[mi * M_BLK:(mi + 1) * M_BLK],
            "b": B[nj * N_BLK:(nj + 1) * N_BLK],
        })
    res = run_bass_kernel_spmd(nc, in_maps, list(range(N_CORES)), trace=trace)
    cost = np.empty((N, N), dtype=np.float32)
    for k in range(N_CORES):
        mi, nj = k // 2, k % 2
        cost[mi * M_BLK:(mi + 1) * M_BLK,
             nj * N_BLK:(nj + 1) * N_BLK] = res.results[k]["cost"]
    return cost, res


def _lap_jv(cost):
    """Exact Jonker-Volgenant LAP (dual potentials + shortest augmenting
    path); identical algorithm to the reference / scipy."""
    cost = np.asarray(cost, dtype=np.float64)
    n = cost.shape[0]
    INF = np.inf
    u = np.zeros(n + 1)
    v = np.zeros(n + 1)
    p = np.zeros(n + 1, dtype=np.int64)
    way = np.zeros(n + 1, dtype=np.int64)
    for i in range(1, n + 1):
        p[0] = i
        j0 = 0
        minv = np.full(n + 1, INF)
        used = np.zeros(n + 1, dtype=bool)
        while True:
            used[j0] = True
            i0 = p[j0]
            cur = cost[i0 - 1, :] - u[i0] - v[1:]
            free = ~used[1:]
            upd = free & (cur < minv[1:])
            minv[1:][upd] = cur[upd]
            way[1:][upd] = j0
            m = np.where(free, minv[1:], INF)
            j1 = int(np.argmin(m)) + 1
            delta = m[j1 - 1]
            iu = np.nonzero(used)[0]
            u[p[iu]] += delta
            v[iu] -= delta
            minv[1:][free] -= delta
            j0 = j1
            if p[j0] == 0:
                break
        while j0 != 0:
            j1 = way[j0]
            p[j0] = p[j1]
            j0 = j1
    col_of_row = np.empty(n, dtype=np.int64)
    col_of_row[p[1:] - 1] = np.arange(n)
    return col_of_row


def _solve_lap(cost):
    try:
        from scipy.optimize import linear_sum_assignment
        _, col = linear_sum_assignment(np.asarray(cost, dtype=np.float64))
        return col
    except Exception:
        return _lap_jv(cost)


def kernel(latent):
    lat = np.asarray(latent)
    cost, _ = _run_device_cost(lat)
    c64 = cost.astype(np.float64)
    col_ind = _solve_lap(c64)
    predicted = c64[np.arange(N), col_ind].sum()
    ideal = np.trace(c64)
    loss = np.float32(abs(predicted - ideal) / N)
    row_ind = np.arange(N, dtype=np.int32)
    return loss, row_ind, col_ind.astype(np.int32)
